# revision 8
# baseline (speedup 1.0000x reference)
"""MultiHeadAttention Trainium2 kernel.

Full inputs: x [4, 2048, 768] f32, W_qkv [2304, 768], W_proj [768, 768],
b_proj [768]. Output [4, 2048, 768] f32.

Sharding: 8 cores = 4 batches x 2 head-groups (6 heads each).
Per-core inputs (host-prepared, transposed on host):
  xT  [768, 2048]  = x[b].T
  wT  [768, 1152]  = concat(Wq_g, Wk_g, Wv_g).T   (g = head group rows)
  wpT [384, 768]   = W_proj[:, g-cols].T
Per-core output: outp [2048, 768] = partial projection output for batch b.
Host: out[b] = outp[2b] + outp[2b+1] + b_proj.

On-device (per core):
  phase 1: qT/kT [384, 2048] (head-dim on partitions) and v [2048, 384+ones]
           via f32r matmuls; x.T and W.T arrive pre-transposed from host.
  phase 2: per (head, k-chunk): energyT[k,q] = kT.T @ qT (K=64), one exp
           activation over 4 psum banks (scale=1/8 folded in, no max
           subtraction -- energies are O(+-10) for this distribution), then
           av[65, q] += v_aug.T @ e accumulated over k-chunks. Row 64 of av
           is the softmax denominator (ones column of v_aug).
           attT[hd, q] = av[0:64] * (1/l broadcast).
  phase 3: out[s, e] = attT.T @ wpT accumulated over hd-chunks -> DMA out.
"""

import numpy as np

import concourse.bass as bass
import concourse.tile as tile
from concourse import bacc, mybir
from concourse.bass_utils import run_bass_kernel_spmd

EMB = 768
N = 2048
B = 4
D = 64
HL = 6            # heads per core
HD = HL * D       # 384 local head-dim columns
NCORES = 8
SCALE = D ** -0.5

F32 = mybir.dt.float32
F32R = mybir.dt.float32r

EC = EMB // 128   # 6 emb chunks
MC = HD // 128    # 3 local head-dim chunks
NQ = N // 512     # 4 query chunks of 512
NK = N // 128     # 16 key/seq chunks of 128

EXP = mybir.ActivationFunctionType.Exp


def _emit(tc):
    from contextlib import ExitStack

    nc = tc.nc
    xT = nc.dram_tensor("xT", [EMB, N], F32R, kind="ExternalInput").ap()
    wT = nc.dram_tensor("wT", [EMB, 3 * HD], F32R, kind="ExternalInput").ap()
    wpT = nc.dram_tensor("wpT", [HD, EMB], F32R, kind="ExternalInput").ap()
    outp = nc.dram_tensor("outp", [N, EMB], F32, kind="ExternalOutput").ap()

    xTr = xT.rearrange("(c p) s -> p c s", p=128)
    wTr = wT.rearrange("(c p) s -> p c s", p=128)
    wpTr = wpT.rearrange("(m p) e -> p m e", p=128)
    outr = outp.rearrange("(s p) e -> p s e", p=128)

    with ExitStack() as persist:
        ppool = persist.enter_context(tc.tile_pool(name="persist", bufs=1))
        wp_sb = ppool.tile([128, MC, EMB], F32R)
        nc.sync.dma_start(wp_sb[:], wpTr)
        qT_sb = ppool.tile([128, MC, N], F32R)
        kT_sb = ppool.tile([128, MC, N], F32R)
        v_sb = ppool.tile([128, NK, HL, D + 1], F32R)
        attT_sb = ppool.tile([128, MC, N], F32R)
        # ones column per head: the AV matmul then also produces the softmax
        # denominator (sum over k of exp) in row D of each av tile.
        # (memset can't target f32r -- stage in f32 and round-copy over)
        ones_sb = ppool.tile([128, NK * HL], F32)
        nc.vector.memset(ones_sb[:], 1.0)
        nc.vector.tensor_copy(v_sb[:, :, :, D:D + 1], ones_sb[:])

        # ---- phase 1: qkv projection ----
        with ExitStack() as ph1:
            p1 = ph1.enter_context(tc.tile_pool(name="ph1", bufs=1))
            x_sb = p1.tile([128, EC, N], F32R)
            nc.sync.dma_start(x_sb[:], xTr)
            w_sb = p1.tile([128, EC, 3 * HD], F32R)
            nc.sync.dma_start(w_sb[:], wTr)
            ps1 = ph1.enter_context(tc.tile_pool(name="ps1", bufs=4, space="PSUM"))

            for which, dst in ((0, qT_sb), (1, kT_sb)):
                for m in range(MC):
                    lo = which * HD + m * 128
                    for n in range(NQ):
                        mm = ps1.tile([128, 512], F32, tag="mm", name=f"mm_{which}_{m}_{n}")
                        for c in range(EC):
                            nc.tensor.matmul(
                                mm[:],
                                (w_sb[:, c, lo:lo + 128]),
                                (x_sb[:, c, n * 512:(n + 1) * 512]),
                                start=(c == 0), stop=(c == EC - 1))
                        nc.vector.tensor_copy(dst[:, m, n * 512:(n + 1) * 512], mm[:])

            for s in range(NK):
                vv = ps1.tile([128, HD], F32, tag="vv", name=f"vv_{s}")
                for c in range(EC):
                    nc.tensor.matmul(
                        vv[:],
                        (x_sb[:, c, s * 128:(s + 1) * 128]),
                        (w_sb[:, c, 2 * HD:3 * HD]),
                        start=(c == 0), stop=(c == EC - 1))
                nc.vector.tensor_copy(
                    v_sb[:, s, :, 0:D],
                    vv[:].rearrange("p (h d) -> p h d", h=HL))

        # ---- phase 2: attention ----
        with ExitStack() as ph2:
            eps_pool = ph2.enter_context(tc.tile_pool(name="eps", bufs=1, space="PSUM"))
            avp_pool = ph2.enter_context(tc.tile_pool(name="avp", bufs=4, space="PSUM"))
            esb_pool = ph2.enter_context(tc.tile_pool(name="esb", bufs=3))
            sm_pool = ph2.enter_context(tc.tile_pool(name="sm", bufs=4))

            for h in range(HL):
                m, p0 = h // 2, (h % 2) * 64
                avs = [avp_pool.tile([D + 1, 512], F32, tag="av", name=f"av_{h}_{n}")
                       for n in range(NQ)]
                for kk in range(NK):
                    e_ps = eps_pool.tile([128, NQ, 512], F32, tag="eps", name=f"eps_{h}_{kk}")
                    for n in range(NQ):
                        nc.tensor.matmul(
                            e_ps[:, n, :],
                            (kT_sb[p0:p0 + 64, m, kk * 128:(kk + 1) * 128]),
                            (qT_sb[p0:p0 + 64, m, n * 512:(n + 1) * 512]),
                            start=True, stop=True)
                    e_sb = esb_pool.tile([128, NQ, 512], F32R, tag="esb", name=f"esb_{h}_{kk}")
                    nc.scalar.activation(e_sb[:], e_ps[:], EXP, scale=SCALE)
                    for n in range(NQ):
                        nc.tensor.matmul(
                            avs[n][:],
                            (v_sb[:, kk, h, :]),
                            (e_sb[:, n, :]),
                            start=(kk == 0), stop=(kk == NK - 1))
                for n in range(NQ):
                    rec = sm_pool.tile([1, 512], F32, tag="rec", name=f"rec_{h}_{n}")
                    nc.vector.reciprocal(rec[:], avs[n][D:D + 1, :])
                    rb = sm_pool.tile([D, 512], F32, tag="rb", name=f"rb_{h}_{n}")
                    nc.gpsimd.partition_broadcast(rb[:], rec[:])
                    nc.vector.tensor_mul(
                        attT_sb[p0:p0 + 64, m, n * 512:(n + 1) * 512],
                        avs[n][0:D, :], rb[:])

        # ---- phase 3: output projection (natural layout) ----
        with ExitStack() as ph3:
            ps3 = ph3.enter_context(tc.tile_pool(name="ps3", bufs=4, space="PSUM"))
            osb_pool = ph3.enter_context(tc.tile_pool(name="osb", bufs=3))
            for s in range(NK):
                o_sb = osb_pool.tile([128, EMB], F32, tag="osb", name=f"osb_{s}")
                for half in range(2):
                    pr = ps3.tile([128, HD], F32, tag="pr", name=f"pr_{s}_{half}")
                    for m in range(MC):
                        nc.tensor.matmul(
                            pr[:],
                            (attT_sb[:, m, s * 128:(s + 1) * 128]),
                            (wp_sb[:, m, half * HD:(half + 1) * HD]),
                            start=(m == 0), stop=(m == MC - 1))
                    nc.vector.tensor_copy(o_sb[:, half * HD:(half + 1) * HD], pr[:])
                nc.sync.dma_start(outr[:, s, :], o_sb[:])


_CACHE = {}


def _build():
    if "nc" not in _CACHE:
        nc = bacc.Bacc("TRN2", target_bir_lowering=False, debug=False,
                       num_devices=NCORES)
        with tile.TileContext(nc) as tc:
            _emit(tc)
        nc.compile()
        _CACHE["nc"] = nc
    return _CACHE["nc"]


def _in_maps(x, W_qkv, W_proj):
    in_maps = []
    for c in range(NCORES):
        b, g = divmod(c, 2)
        r0 = g * HD
        w_rows = np.concatenate([
            W_qkv[0 * EMB + r0: 0 * EMB + r0 + HD],
            W_qkv[1 * EMB + r0: 1 * EMB + r0 + HD],
            W_qkv[2 * EMB + r0: 2 * EMB + r0 + HD],
        ], axis=0)                                   # [1152, 768]
        in_maps.append({
            "xT": np.ascontiguousarray(x[b].T),
            "wT": np.ascontiguousarray(w_rows.T),
            "wpT": np.ascontiguousarray(W_proj[:, r0:r0 + HD].T),
        })
    return in_maps


LAST_RESULTS = None


def kernel(x, W_qkv, W_proj, b_proj):
    global LAST_RESULTS
    x = np.ascontiguousarray(np.asarray(x, dtype=np.float32))
    W_qkv = np.asarray(W_qkv, dtype=np.float32)
    W_proj = np.asarray(W_proj, dtype=np.float32)
    b_proj = np.asarray(b_proj, dtype=np.float32)

    nc = _build()
    in_maps = _in_maps(x, W_qkv, W_proj)
    res = run_bass_kernel_spmd(nc, in_maps, core_ids=list(range(NCORES)))
    LAST_RESULTS = res

    out = np.empty((B, N, EMB), dtype=np.float32)
    for b in range(B):
        out[b] = res.results[2 * b]["outp"] + res.results[2 * b + 1]["outp"]
    out += b_proj
    return out


# revision 11
# speedup vs baseline: 1.0765x; 1.0765x over previous
"""MultiHeadAttention Trainium2 kernel.

Full inputs: x [4, 2048, 768] f32, W_qkv [2304, 768], W_proj [768, 768],
b_proj [768]. Output [4, 2048, 768] f32.

Sharding: 8 cores = 4 batches x 2 head-groups (6 heads each).
Per-core inputs (host-prepared, transposed on host):
  xT  [768, 2048]  = x[b].T
  wT  [768, 1152]  = concat(Wq_g, Wk_g, Wv_g).T   (g = head group rows)
  wpT [384, 768]   = W_proj[:, g-cols].T
Per-core output: outp [2048, 768] = partial projection output for batch b.
Host: out[b] = outp[2b] + outp[2b+1] + b_proj.

On-device (per core):
  phase 1: qT/kT [384, 2048] (head-dim on partitions) and v [2048, 384+ones]
           via f32r matmuls; x.T and W.T arrive pre-transposed from host.
  phase 2: per (head, k-chunk): energyT[k,q] = kT.T @ qT (K=64), one exp
           activation over 4 psum banks (scale=1/8 folded in, no max
           subtraction -- energies are O(+-10) for this distribution), then
           av[65, q] += v_aug.T @ e accumulated over k-chunks. Row 64 of av
           is the softmax denominator (ones column of v_aug).
           attT[hd, q] = av[0:64] * (1/l broadcast).
  phase 3: out[s, e] = attT.T @ wpT accumulated over hd-chunks -> DMA out.
"""

import ml_dtypes
import numpy as np

import concourse.bass as bass
import concourse.tile as tile
from concourse import bacc, mybir
from concourse.bass_utils import run_bass_kernel_spmd

EMB = 768
N = 2048
B = 4
D = 64
HL = 6            # heads per core
HD = HL * D       # 384 local head-dim columns
NCORES = 8
SCALE = D ** -0.5

F32 = mybir.dt.float32
BF16 = mybir.dt.bfloat16

EC = EMB // 128   # 6 emb chunks
MC = HD // 128    # 3 local head-dim chunks
NQ = N // 512     # 4 query chunks of 512
NK = N // 128     # 16 key/seq chunks of 128

EXP = mybir.ActivationFunctionType.Exp


def _emit(tc):
    from contextlib import ExitStack

    nc = tc.nc
    xT = nc.dram_tensor("xT", [EMB, N], BF16, kind="ExternalInput").ap()
    wT = nc.dram_tensor("wT", [EMB, 3 * HD], BF16, kind="ExternalInput").ap()
    wpT = nc.dram_tensor("wpT", [HD, EMB], BF16, kind="ExternalInput").ap()
    outp = nc.dram_tensor("outp", [N, EMB], F32, kind="ExternalOutput").ap()

    xTr = xT.rearrange("(c p) s -> p c s", p=128)
    wTr = wT.rearrange("(c p) s -> p c s", p=128)
    wpTr = wpT.rearrange("(m p) e -> p m e", p=128)
    outr = outp.rearrange("(s p) e -> p s e", p=128)

    with ExitStack() as persist:
        ppool = persist.enter_context(tc.tile_pool(name="persist", bufs=1))
        wp_sb = ppool.tile([128, MC, EMB], BF16)
        nc.sync.dma_start(wp_sb[:], wpTr)
        qT_sb = ppool.tile([128, MC, N], BF16)
        kT_sb = ppool.tile([128, MC, N], BF16)
        v_sb = ppool.tile([128, NK, HL, D + 1], BF16)
        attT_sb = ppool.tile([128, MC, N], BF16)
        # ones column per head: the AV matmul then also produces the softmax
        # denominator (sum over k of exp) in row D of each av tile.
        # (memset can't target f32r -- stage in f32 and round-copy over)
        ones_sb = ppool.tile([128, NK * HL], BF16)
        nc.vector.memset(ones_sb[:], 1.0)
        nc.vector.tensor_copy(v_sb[:, :, :, D:D + 1], ones_sb[:])

        # ---- phase 1: qkv projection ----
        with ExitStack() as ph1:
            p1 = ph1.enter_context(tc.tile_pool(name="ph1", bufs=1))
            x_sb = p1.tile([128, EC, N], BF16)
            w_sb = p1.tile([128, EC, 3 * HD], BF16)
            for c in range(EC):
                nc.sync.dma_start(w_sb[:, c, :], wTr[:, c, :])
                nc.sync.dma_start(x_sb[:, c, :], xTr[:, c, :])
            ps1 = ph1.enter_context(tc.tile_pool(name="ps1", bufs=4, space="PSUM"))

            for which, dst in ((0, qT_sb), (1, kT_sb)):
                for m in range(MC):
                    lo = which * HD + m * 128
                    for n in range(NQ):
                        mm = ps1.tile([128, 512], F32, tag="mm", name=f"mm_{which}_{m}_{n}")
                        for c in range(EC):
                            nc.tensor.matmul(
                                mm[:],
                                (w_sb[:, c, lo:lo + 128]),
                                (x_sb[:, c, n * 512:(n + 1) * 512]),
                                start=(c == 0), stop=(c == EC - 1))
                        nc.vector.tensor_copy(dst[:, m, n * 512:(n + 1) * 512], mm[:])

            for s in range(NK):
                vv = ps1.tile([128, HD], F32, tag="vv", name=f"vv_{s}")
                for c in range(EC):
                    nc.tensor.matmul(
                        vv[:],
                        (x_sb[:, c, s * 128:(s + 1) * 128]),
                        (w_sb[:, c, 2 * HD:3 * HD]),
                        start=(c == 0), stop=(c == EC - 1))
                nc.vector.tensor_copy(
                    v_sb[:, s, :, 0:D],
                    vv[:].rearrange("p (h d) -> p h d", h=HL))

        # ---- phase 2: attention ----
        with ExitStack() as ph2:
            eps_pool = ph2.enter_context(tc.tile_pool(name="eps", bufs=1, space="PSUM"))
            avp_pool = ph2.enter_context(tc.tile_pool(name="avp", bufs=4, space="PSUM"))
            esb_pool = ph2.enter_context(tc.tile_pool(name="esb", bufs=3))
            sm_pool = ph2.enter_context(tc.tile_pool(name="sm", bufs=4))

            for h in range(HL):
                m, p0 = h // 2, (h % 2) * 64
                avs = [avp_pool.tile([D + 1, 512], F32, tag="av", name=f"av_{h}_{n}")
                       for n in range(NQ)]
                for kk in range(NK):
                    e_ps = eps_pool.tile([128, NQ, 512], F32, tag="eps", name=f"eps_{h}_{kk}")
                    for n in range(NQ):
                        nc.tensor.matmul(
                            e_ps[:, n, :],
                            (kT_sb[p0:p0 + 64, m, kk * 128:(kk + 1) * 128]),
                            (qT_sb[p0:p0 + 64, m, n * 512:(n + 1) * 512]),
                            start=True, stop=True)
                    e_sb = esb_pool.tile([128, NQ, 512], BF16, tag="esb", name=f"esb_{h}_{kk}")
                    nc.scalar.activation(e_sb[:], e_ps[:], EXP, scale=SCALE)
                    for n in range(NQ):
                        nc.tensor.matmul(
                            avs[n][:],
                            (v_sb[:, kk, h, :]),
                            (e_sb[:, n, :]),
                            start=(kk == 0), stop=(kk == NK - 1))
                for n in range(NQ):
                    rec = sm_pool.tile([1, 512], F32, tag="rec", name=f"rec_{h}_{n}")
                    nc.vector.reciprocal(rec[:], avs[n][D:D + 1, :])
                    rb = sm_pool.tile([D, 512], F32, tag="rb", name=f"rb_{h}_{n}")
                    nc.gpsimd.partition_broadcast(rb[:], rec[:])
                    nc.vector.tensor_mul(
                        attT_sb[p0:p0 + 64, m, n * 512:(n + 1) * 512],
                        avs[n][0:D, :], rb[:])

        # ---- phase 3: output projection (natural layout) ----
        with ExitStack() as ph3:
            ps3 = ph3.enter_context(tc.tile_pool(name="ps3", bufs=4, space="PSUM"))
            osb_pool = ph3.enter_context(tc.tile_pool(name="osb", bufs=3))
            for s in range(NK):
                o_sb = osb_pool.tile([128, EMB], F32, tag="osb", name=f"osb_{s}")
                for half in range(2):
                    pr = ps3.tile([128, HD], F32, tag="pr", name=f"pr_{s}_{half}")
                    for m in range(MC):
                        nc.tensor.matmul(
                            pr[:],
                            (attT_sb[:, m, s * 128:(s + 1) * 128]),
                            (wp_sb[:, m, half * HD:(half + 1) * HD]),
                            start=(m == 0), stop=(m == MC - 1))
                    nc.vector.tensor_copy(o_sb[:, half * HD:(half + 1) * HD], pr[:])
                nc.sync.dma_start(outr[:, s, :], o_sb[:])


_CACHE = {}


def _build():
    if "nc" not in _CACHE:
        nc = bacc.Bacc("TRN2", target_bir_lowering=False, debug=False,
                       num_devices=NCORES)
        with tile.TileContext(nc) as tc:
            _emit(tc)
        nc.compile()
        _CACHE["nc"] = nc
    return _CACHE["nc"]


def _in_maps(x, W_qkv, W_proj):
    in_maps = []
    for c in range(NCORES):
        b, g = divmod(c, 2)
        r0 = g * HD
        w_rows = np.concatenate([
            W_qkv[0 * EMB + r0: 0 * EMB + r0 + HD],
            W_qkv[1 * EMB + r0: 1 * EMB + r0 + HD],
            W_qkv[2 * EMB + r0: 2 * EMB + r0 + HD],
        ], axis=0)                                   # [1152, 768]
        bf = ml_dtypes.bfloat16
        in_maps.append({
            "xT": np.ascontiguousarray(x[b].T.astype(bf)),
            "wT": np.ascontiguousarray(w_rows.T.astype(bf)),
            "wpT": np.ascontiguousarray(W_proj[:, r0:r0 + HD].T.astype(bf)),
        })
    return in_maps


LAST_RESULTS = None


def kernel(x, W_qkv, W_proj, b_proj):
    global LAST_RESULTS
    x = np.ascontiguousarray(np.asarray(x, dtype=np.float32))
    W_qkv = np.asarray(W_qkv, dtype=np.float32)
    W_proj = np.asarray(W_proj, dtype=np.float32)
    b_proj = np.asarray(b_proj, dtype=np.float32)

    nc = _build()
    in_maps = _in_maps(x, W_qkv, W_proj)
    res = run_bass_kernel_spmd(nc, in_maps, core_ids=list(range(NCORES)))
    LAST_RESULTS = res

    out = np.empty((B, N, EMB), dtype=np.float32)
    for b in range(B):
        out[b] = res.results[2 * b]["outp"] + res.results[2 * b + 1]["outp"]
    out += b_proj
    return out


# revision 13
# speedup vs baseline: 1.5315x; 1.4227x over previous
"""MultiHeadAttention Trainium2 kernel.

Full inputs: x [4, 2048, 768] f32, W_qkv [2304, 768], W_proj [768, 768],
b_proj [768]. Output [4, 2048, 768] f32.

Sharding: 8 cores = 4 batches x 2 head-groups (6 heads each).
Per-core inputs (host-prepared, transposed on host):
  xT  [768, 2048]  = x[b].T
  wT  [768, 1152]  = concat(Wq_g, Wk_g, Wv_g).T   (g = head group rows)
  wpT [384, 768]   = W_proj[:, g-cols].T
Per-core output: outp [2048, 768] = partial projection output for batch b.
Host: out[b] = outp[2b] + outp[2b+1] + b_proj.

On-device (per core):
  phase 1: qT/kT [384, 2048] (head-dim on partitions) and v [2048, 384+ones]
           via f32r matmuls; x.T and W.T arrive pre-transposed from host.
  phase 2: per (head, k-chunk): energyT[k,q] = kT.T @ qT (K=64), one exp
           activation over 4 psum banks (scale=1/8 folded in, no max
           subtraction -- energies are O(+-10) for this distribution), then
           av[65, q] += v_aug.T @ e accumulated over k-chunks. Row 64 of av
           is the softmax denominator (ones column of v_aug).
           attT[hd, q] = av[0:64] * (1/l broadcast).
  phase 3: out[s, e] = attT.T @ wpT accumulated over hd-chunks -> DMA out.
"""

import ml_dtypes
import numpy as np

import concourse.bass as bass
import concourse.tile as tile
from concourse import bacc, mybir
from concourse.bass_utils import run_bass_kernel_spmd

EMB = 768
N = 2048
B = 4
D = 64
HL = 6            # heads per core
HD = HL * D       # 384 local head-dim columns
NCORES = 8
SCALE = D ** -0.5

F32 = mybir.dt.float32
BF16 = mybir.dt.bfloat16

EC = EMB // 128   # 6 emb chunks
MC = HD // 128    # 3 local head-dim chunks
NQ = N // 512     # 4 query chunks of 512
NK = N // 128     # 16 key/seq chunks of 128

EXP = mybir.ActivationFunctionType.Exp


def _emit(tc):
    from contextlib import ExitStack

    nc = tc.nc
    xT = nc.dram_tensor("xT", [EMB, N], BF16, kind="ExternalInput").ap()
    wT = nc.dram_tensor("wT", [EMB, 3 * HD], BF16, kind="ExternalInput").ap()
    wpT = nc.dram_tensor("wpT", [HD, EMB], BF16, kind="ExternalInput").ap()
    outp = nc.dram_tensor("outp", [N, EMB], F32, kind="ExternalOutput").ap()

    xTr = xT.rearrange("(c p) s -> p c s", p=128)
    wTr = wT.rearrange("(c p) s -> p c s", p=128)
    wpTr = wpT.rearrange("(m p) e -> p m e", p=128)
    outr = outp.rearrange("(s p) e -> p s e", p=128)

    with ExitStack() as persist:
        ppool = persist.enter_context(tc.tile_pool(name="persist", bufs=1))
        wp_sb = ppool.tile([128, MC, EMB], BF16)
        nc.sync.dma_start(wp_sb[:], wpTr)
        qT_sb = ppool.tile([128, MC, N], BF16)
        kT_sb = ppool.tile([128, MC, N], BF16)
        v_sb = ppool.tile([128, NK, HL, D + 1], BF16)
        attT_sb = ppool.tile([128, MC, N], BF16)
        # ones column per head: the AV matmul then also produces the softmax
        # denominator (sum over k of exp) in row D of each av tile.
        # (memset can't target f32r -- stage in f32 and round-copy over)
        ones_sb = ppool.tile([128, NK * HL], BF16)
        nc.vector.memset(ones_sb[:], 1.0)
        nc.vector.tensor_copy(v_sb[:, :, :, D:D + 1], ones_sb[:])

        # ---- phase 1: qkv projection ----
        with ExitStack() as ph1:
            p1 = ph1.enter_context(tc.tile_pool(name="ph1", bufs=1))
            x_sb = p1.tile([128, EC, N], BF16)
            w_sb = p1.tile([128, EC, 3 * HD], BF16)
            for c in range(EC):
                nc.sync.dma_start(w_sb[:, c, :], wTr[:, c, :])
                nc.sync.dma_start(x_sb[:, c, :], xTr[:, c, :])
            ps1 = ph1.enter_context(tc.tile_pool(name="ps1", bufs=4, space="PSUM"))

            for which, dst in ((0, qT_sb), (1, kT_sb)):
                for m in range(MC):
                    lo = which * HD + m * 128
                    for n in range(NQ):
                        mm = ps1.tile([128, 512], F32, tag="mm", name=f"mm_{which}_{m}_{n}")
                        for c in range(EC):
                            nc.tensor.matmul(
                                mm[:],
                                (w_sb[:, c, lo:lo + 128]),
                                (x_sb[:, c, n * 512:(n + 1) * 512]),
                                start=(c == 0), stop=(c == EC - 1))
                        nc.vector.tensor_copy(dst[:, m, n * 512:(n + 1) * 512], mm[:])

            for s in range(NK):
                vv = ps1.tile([128, HD], F32, tag="vv", name=f"vv_{s}")
                for c in range(EC):
                    nc.tensor.matmul(
                        vv[:],
                        (x_sb[:, c, s * 128:(s + 1) * 128]),
                        (w_sb[:, c, 2 * HD:3 * HD]),
                        start=(c == 0), stop=(c == EC - 1))
                nc.vector.tensor_copy(
                    v_sb[:, s, :, 0:D],
                    vv[:].rearrange("p (h d) -> p h d", h=HL))

        # ---- phase 2: attention ----
        with ExitStack() as ph2:
            eps_pool = ph2.enter_context(tc.tile_pool(name="eps", bufs=2, space="PSUM"))
            avp_pool = ph2.enter_context(tc.tile_pool(name="avp", bufs=4, space="PSUM"))
            esb_pool = ph2.enter_context(tc.tile_pool(name="esb", bufs=4))
            sm_pool = ph2.enter_context(tc.tile_pool(name="sm", bufs=4))

            for h in range(HL):
                m, p0 = h // 2, (h % 2) * 64
                avs = [avp_pool.tile([D + 1, 512], F32, tag="av", name=f"av_{h}_{n}")
                       for n in range(NQ)]
                for kk in range(NK):
                    # two 2-bank energy tiles per kk so the next group's QK
                    # overlaps this group's exp (keeps the PE array gap-free;
                    # periodic array idles re-throttle the HAM clock gate)
                    e_sbs = []
                    for half in range(2):
                        e_ps = eps_pool.tile([128, 2, 512], F32, tag="eps",
                                             name=f"eps_{h}_{kk}_{half}")
                        for j in range(2):
                            n = half * 2 + j
                            nc.tensor.matmul(
                                e_ps[:, j, :],
                                (kT_sb[p0:p0 + 64, m, kk * 128:(kk + 1) * 128]),
                                (qT_sb[p0:p0 + 64, m, n * 512:(n + 1) * 512]),
                                start=True, stop=True)
                        e_sb = esb_pool.tile([128, 2, 512], BF16, tag="esb",
                                             name=f"esb_{h}_{kk}_{half}")
                        nc.scalar.activation(e_sb[:], e_ps[:], EXP, scale=SCALE)
                        e_sbs.append(e_sb)
                    for n in range(NQ):
                        nc.tensor.matmul(
                            avs[n][:],
                            (v_sb[:, kk, h, :]),
                            (e_sbs[n // 2][:, n % 2, :]),
                            start=(kk == 0), stop=(kk == NK - 1))
                for n in range(NQ):
                    # drain the psum bank right away so the (slow) reciprocal
                    # chain never blocks the next head's AV accumulation
                    avst = sm_pool.tile([D + 1, 512], F32, tag="avst",
                                        name=f"avst_{h}_{n}")
                    nc.vector.tensor_copy(avst[:], avs[n][:])
                    rec = sm_pool.tile([1, 512], F32, tag="rec", name=f"rec_{h}_{n}")
                    nc.vector.reciprocal(rec[:], avst[D:D + 1, :])
                    rb = sm_pool.tile([D, 512], F32, tag="rb", name=f"rb_{h}_{n}")
                    nc.gpsimd.partition_broadcast(rb[:], rec[:])
                    nc.vector.tensor_mul(
                        attT_sb[p0:p0 + 64, m, n * 512:(n + 1) * 512],
                        avst[0:D, :], rb[:])

        # ---- phase 3: output projection (natural layout) ----
        with ExitStack() as ph3:
            ps3 = ph3.enter_context(tc.tile_pool(name="ps3", bufs=4, space="PSUM"))
            osb_pool = ph3.enter_context(tc.tile_pool(name="osb", bufs=3))
            for s in range(NK):
                o_sb = osb_pool.tile([128, EMB], F32, tag="osb", name=f"osb_{s}")
                for half in range(2):
                    pr = ps3.tile([128, HD], F32, tag="pr", name=f"pr_{s}_{half}")
                    for m in range(MC):
                        nc.tensor.matmul(
                            pr[:],
                            (attT_sb[:, m, s * 128:(s + 1) * 128]),
                            (wp_sb[:, m, half * HD:(half + 1) * HD]),
                            start=(m == 0), stop=(m == MC - 1))
                    nc.vector.tensor_copy(o_sb[:, half * HD:(half + 1) * HD], pr[:])
                nc.sync.dma_start(outr[:, s, :], o_sb[:])


_CACHE = {}


def _build():
    if "nc" not in _CACHE:
        nc = bacc.Bacc("TRN2", target_bir_lowering=False, debug=False,
                       num_devices=NCORES)
        with tile.TileContext(nc) as tc:
            _emit(tc)
        nc.compile()
        _CACHE["nc"] = nc
    return _CACHE["nc"]


def _in_maps(x, W_qkv, W_proj):
    in_maps = []
    for c in range(NCORES):
        b, g = divmod(c, 2)
        r0 = g * HD
        w_rows = np.concatenate([
            W_qkv[0 * EMB + r0: 0 * EMB + r0 + HD],
            W_qkv[1 * EMB + r0: 1 * EMB + r0 + HD],
            W_qkv[2 * EMB + r0: 2 * EMB + r0 + HD],
        ], axis=0)                                   # [1152, 768]
        bf = ml_dtypes.bfloat16
        in_maps.append({
            "xT": np.ascontiguousarray(x[b].T.astype(bf)),
            "wT": np.ascontiguousarray(w_rows.T.astype(bf)),
            "wpT": np.ascontiguousarray(W_proj[:, r0:r0 + HD].T.astype(bf)),
        })
    return in_maps


LAST_RESULTS = None


def kernel(x, W_qkv, W_proj, b_proj):
    global LAST_RESULTS
    x = np.ascontiguousarray(np.asarray(x, dtype=np.float32))
    W_qkv = np.asarray(W_qkv, dtype=np.float32)
    W_proj = np.asarray(W_proj, dtype=np.float32)
    b_proj = np.asarray(b_proj, dtype=np.float32)

    nc = _build()
    in_maps = _in_maps(x, W_qkv, W_proj)
    res = run_bass_kernel_spmd(nc, in_maps, core_ids=list(range(NCORES)))
    LAST_RESULTS = res

    out = np.empty((B, N, EMB), dtype=np.float32)
    for b in range(B):
        out[b] = res.results[2 * b]["outp"] + res.results[2 * b + 1]["outp"]
    out += b_proj
    return out


# revision 18
# speedup vs baseline: 1.9809x; 1.2934x over previous
"""MultiHeadAttention Trainium2 kernel.

Full inputs: x [4, 2048, 768] f32, W_qkv [2304, 768], W_proj [768, 768],
b_proj [768]. Output [4, 2048, 768] f32.

Sharding: 8 cores = 4 batches x 2 head-groups (6 heads each).
Per-core inputs (host-prepared, transposed on host):
  xT  [768, 2048]  = x[b].T
  wT  [768, 1152]  = concat(Wq_g, Wk_g, Wv_g).T   (g = head group rows)
  wpT [384, 768]   = W_proj[:, g-cols].T
Per-core output: outp [2048, 768] = partial projection output for batch b.
Host: out[b] = outp[2b] + outp[2b+1] + b_proj.

On-device (per core):
  phase 1: qT/kT [384, 2048] (head-dim on partitions) and v [2048, 384+ones]
           via f32r matmuls; x.T and W.T arrive pre-transposed from host.
  phase 2: per (head, k-chunk): energyT[k,q] = kT.T @ qT (K=64), one exp
           activation over 4 psum banks (scale=1/8 folded in, no max
           subtraction -- energies are O(+-10) for this distribution), then
           av[65, q] += v_aug.T @ e accumulated over k-chunks. Row 64 of av
           is the softmax denominator (ones column of v_aug).
           attT[hd, q] = av[0:64] * (1/l broadcast).
  phase 3: out[s, e] = attT.T @ wpT accumulated over hd-chunks -> DMA out.
"""

import ml_dtypes
import numpy as np

import concourse.bass as bass
import concourse.tile as tile
from concourse import bacc, mybir
from concourse.bass_utils import run_bass_kernel_spmd

EMB = 768
N = 2048
B = 4
D = 64
HL = 6            # heads per core
HD = HL * D       # 384 local head-dim columns
NCORES = 8
SCALE = D ** -0.5

F32 = mybir.dt.float32
BF16 = mybir.dt.bfloat16

EC = EMB // 128   # 6 emb chunks
MC = HD // 128    # 3 local head-dim chunks
NQ = N // 512     # 4 query chunks of 512
NK = N // 128     # 16 key/seq chunks of 128

EXP = mybir.ActivationFunctionType.Exp


def _emit(tc):
    from contextlib import ExitStack

    nc = tc.nc
    xT = nc.dram_tensor("xT", [EMB, N], BF16, kind="ExternalInput").ap()
    wT = nc.dram_tensor("wT", [EMB, 3 * HD], BF16, kind="ExternalInput").ap()
    wpT = nc.dram_tensor("wpT", [HD, EMB], BF16, kind="ExternalInput").ap()
    outp = nc.dram_tensor("outp", [N, EMB], F32, kind="ExternalOutput").ap()

    xTr = xT.rearrange("(c p) s -> p c s", p=128)
    wTr = wT.rearrange("(c p) s -> p c s", p=128)
    wpTr = wpT.rearrange("(m p) e -> p m e", p=128)
    outr = outp.rearrange("(s p) e -> p s e", p=128)

    with ExitStack() as persist:
        ppool = persist.enter_context(tc.tile_pool(name="persist", bufs=1))
        wp_sb = ppool.tile([128, MC, EMB], BF16)
        nc.sync.dma_start(wp_sb[:], wpTr)
        qT_sb = ppool.tile([128, MC, N], BF16)
        kT_sb = ppool.tile([128, HL, N], BF16)
        nc.vector.memset(kT_sb[:], 0.0)
        v_sb = ppool.tile([128, NK, HL * (D + 1) + D], BF16)
        nc.vector.memset(v_sb[:], 1.0)
        attT_sb = ppool.tile([128, MC, N], BF16)

        # ---- phase 1: qkv projection ----
        with ExitStack() as ph1:
            p1 = ph1.enter_context(tc.tile_pool(name="ph1", bufs=1))
            x_sb = p1.tile([128, EC, N], BF16)
            w_sb = p1.tile([128, EC, 3 * HD], BF16)
            for c in range(EC):
                nc.sync.dma_start(w_sb[:, c, :], wTr[:, c, :])
                nc.sync.dma_start(x_sb[:, c, :], xTr[:, c, :])
            ps1 = ph1.enter_context(tc.tile_pool(name="ps1", bufs=4, space="PSUM"))

            for which in (0, 1):
                for m in range(MC):
                    lo = which * HD + m * 128
                    for n in range(NQ):
                        mm = ps1.tile([128, 512], F32, tag="mm", name=f"mm_{which}_{m}_{n}")
                        for c in range(EC):
                            nc.tensor.matmul(
                                mm[:],
                                (w_sb[:, c, lo:lo + 128]),
                                (x_sb[:, c, n * 512:(n + 1) * 512]),
                                start=(c == 0), stop=(c == EC - 1))
                        ns = slice(n * 512, (n + 1) * 512)
                        if which == 0:
                            nc.vector.tensor_copy(qT_sb[:, m, ns], mm[:])
                        else:
                            nc.vector.tensor_copy(kT_sb[0:64, 2 * m, ns], mm[0:64, :])
                            nc.vector.tensor_copy(kT_sb[64:128, 2 * m + 1, ns], mm[64:128, :])

            for s in range(NK):
                vv = ps1.tile([128, HD], F32, tag="vv", name=f"vv_{s}")
                for c in range(EC):
                    nc.tensor.matmul(
                        vv[:],
                        (x_sb[:, c, s * 128:(s + 1) * 128]),
                        (w_sb[:, c, 2 * HD:3 * HD]),
                        start=(c == 0), stop=(c == EC - 1))
                nc.vector.tensor_copy(
                    v_sb[:, s, 0:HL * (D + 1)].rearrange(
                        "p (h c) -> p h c", c=D + 1)[:, :, 0:D],
                    vv[:].rearrange("p (h d) -> p h d", h=HL))

        # ---- phase 2: attention ----
        with ExitStack() as ph2:
            eps_pool = ph2.enter_context(tc.tile_pool(name="eps", bufs=2, space="PSUM"))
            avp_pool = ph2.enter_context(tc.tile_pool(name="avp", bufs=4, space="PSUM"))
            esb_pool = ph2.enter_context(tc.tile_pool(name="esb", bufs=4))
            sm_pool = ph2.enter_context(tc.tile_pool(name="sm", bufs=4))

            for h in range(HL):
                m, p0 = h // 2, (h % 2) * 64
                avs = [avp_pool.tile([128, 512], F32, tag="av", name=f"av_{h}_{n}")
                       for n in range(NQ)]
                for kk in range(NK):
                    # two 2-bank energy tiles per kk so the next group's QK
                    # overlaps this group's exp (keeps the PE array gap-free;
                    # periodic array idles re-throttle the HAM clock gate)
                    e_sbs = []
                    for half in range(2):
                        e_ps = eps_pool.tile([128, 2, 512], F32, tag="eps",
                                             name=f"eps_{h}_{kk}_{half}")
                        for j in range(2):
                            n = half * 2 + j
                            nc.tensor.matmul(
                                e_ps[:, j, :],
                                (kT_sb[:, h, kk * 128:(kk + 1) * 128]),
                                (qT_sb[0:128, m, n * 512:(n + 1) * 512]),
                                start=True, stop=True)
                        e_sb = esb_pool.tile([128, 2, 512], BF16, tag="esb",
                                             name=f"esb_{h}_{kk}_{half}")
                        nc.scalar.activation(e_sb[:], e_ps[:], EXP, scale=SCALE)
                        e_sbs.append(e_sb)
                    for n in range(NQ):
                        nc.tensor.matmul(
                            avs[n][:],
                            (v_sb[:, kk, h * (D + 1): h * (D + 1) + 128]),
                            (e_sbs[n // 2][:, n % 2, :]),
                            start=(kk == 0), stop=(kk == NK - 1))
                for n in range(NQ):
                    # drain the psum bank right away so the (slow) reciprocal
                    # chain never blocks the next head's AV accumulation
                    avst = sm_pool.tile([D + 1, 512], F32, tag="avst",
                                        name=f"avst_{h}_{n}")
                    nc.vector.tensor_copy(avst[:], avs[n][0:D + 1, :])
                    rec = sm_pool.tile([1, 512], F32, tag="rec", name=f"rec_{h}_{n}")
                    nc.vector.reciprocal(rec[:], avst[D:D + 1, :])
                    rb = sm_pool.tile([D, 512], F32, tag="rb", name=f"rb_{h}_{n}")
                    nc.gpsimd.partition_broadcast(rb[:], rec[:])
                    nc.vector.tensor_mul(
                        attT_sb[p0:p0 + 64, m, n * 512:(n + 1) * 512],
                        avst[0:D, :], rb[:])

        # ---- phase 3: output projection (natural layout) ----
        with ExitStack() as ph3:
            ps3 = ph3.enter_context(tc.tile_pool(name="ps3", bufs=4, space="PSUM"))
            osb_pool = ph3.enter_context(tc.tile_pool(name="osb", bufs=3))
            for s in range(NK):
                o_sb = osb_pool.tile([128, EMB], F32, tag="osb", name=f"osb_{s}")
                for half in range(2):
                    pr = ps3.tile([128, HD], F32, tag="pr", name=f"pr_{s}_{half}")
                    for m in range(MC):
                        nc.tensor.matmul(
                            pr[:],
                            (attT_sb[:, m, s * 128:(s + 1) * 128]),
                            (wp_sb[:, m, half * HD:(half + 1) * HD]),
                            start=(m == 0), stop=(m == MC - 1))
                    nc.vector.tensor_copy(o_sb[:, half * HD:(half + 1) * HD], pr[:])
                nc.sync.dma_start(outr[:, s, :], o_sb[:])


_CACHE = {}


def _build():
    if "nc" not in _CACHE:
        nc = bacc.Bacc("TRN2", target_bir_lowering=False, debug=False,
                       num_devices=NCORES)
        with tile.TileContext(nc) as tc:
            _emit(tc)
        nc.compile()
        _CACHE["nc"] = nc
    return _CACHE["nc"]


def _in_maps(x, W_qkv, W_proj):
    in_maps = []
    for c in range(NCORES):
        b, g = divmod(c, 2)
        r0 = g * HD
        w_rows = np.concatenate([
            W_qkv[0 * EMB + r0: 0 * EMB + r0 + HD],
            W_qkv[1 * EMB + r0: 1 * EMB + r0 + HD],
            W_qkv[2 * EMB + r0: 2 * EMB + r0 + HD],
        ], axis=0)                                   # [1152, 768]
        bf = ml_dtypes.bfloat16
        in_maps.append({
            "xT": np.ascontiguousarray(x[b].T.astype(bf)),
            "wT": np.ascontiguousarray(w_rows.T.astype(bf)),
            "wpT": np.ascontiguousarray(W_proj[:, r0:r0 + HD].T.astype(bf)),
        })
    return in_maps


LAST_RESULTS = None


def kernel(x, W_qkv, W_proj, b_proj):
    global LAST_RESULTS
    x = np.ascontiguousarray(np.asarray(x, dtype=np.float32))
    W_qkv = np.asarray(W_qkv, dtype=np.float32)
    W_proj = np.asarray(W_proj, dtype=np.float32)
    b_proj = np.asarray(b_proj, dtype=np.float32)

    nc = _build()
    in_maps = _in_maps(x, W_qkv, W_proj)
    res = run_bass_kernel_spmd(nc, in_maps, core_ids=list(range(NCORES)))
    LAST_RESULTS = res

    out = np.empty((B, N, EMB), dtype=np.float32)
    for b in range(B):
        out[b] = res.results[2 * b]["outp"] + res.results[2 * b + 1]["outp"]
    out += b_proj
    return out


# revision 20
# speedup vs baseline: 2.2591x; 1.1405x over previous
"""MultiHeadAttention Trainium2 kernel.

Full inputs: x [4, 2048, 768] f32, W_qkv [2304, 768], W_proj [768, 768],
b_proj [768]. Output [4, 2048, 768] f32.

Sharding: 8 cores = 4 batches x 2 head-groups (6 heads each).
Per-core inputs (host-prepared, transposed on host):
  xT  [768, 2048]  = x[b].T
  wT  [768, 1152]  = concat(Wq_g, Wk_g, Wv_g).T   (g = head group rows)
  wpT [384, 768]   = W_proj[:, g-cols].T
Per-core output: outp [2048, 768] = partial projection output for batch b.
Host: out[b] = outp[2b] + outp[2b+1] + b_proj.

On-device (per core):
  phase 1: qT/kT [384, 2048] (head-dim on partitions) and v [2048, 384+ones]
           via f32r matmuls; x.T and W.T arrive pre-transposed from host.
  phase 2: per (head, k-chunk): energyT[k,q] = kT.T @ qT (K=64), one exp
           activation over 4 psum banks (scale=1/8 folded in, no max
           subtraction -- energies are O(+-10) for this distribution), then
           av[65, q] += v_aug.T @ e accumulated over k-chunks. Row 64 of av
           is the softmax denominator (ones column of v_aug).
           attT[hd, q] = av[0:64] * (1/l broadcast).
  phase 3: out[s, e] = attT.T @ wpT accumulated over hd-chunks -> DMA out.
"""

import ml_dtypes
import numpy as np

import concourse.bass as bass
import concourse.tile as tile
from concourse import bacc, mybir
from concourse.bass_utils import run_bass_kernel_spmd

EMB = 768
N = 2048
B = 4
D = 64
HL = 6            # heads per core
HD = HL * D       # 384 local head-dim columns
NCORES = 8
SCALE = D ** -0.5

F32 = mybir.dt.float32
BF16 = mybir.dt.bfloat16

EC = EMB // 128   # 6 emb chunks
MC = HD // 128    # 3 local head-dim chunks
NQ = N // 512     # 4 query chunks of 512
NK = N // 128     # 16 key/seq chunks of 128

EXP = mybir.ActivationFunctionType.Exp


def _emit(tc):
    from contextlib import ExitStack

    nc = tc.nc
    xT = nc.dram_tensor("xT", [EMB, N], BF16, kind="ExternalInput").ap()
    wT = nc.dram_tensor("wT", [EMB, 3 * HD], BF16, kind="ExternalInput").ap()
    wpT = nc.dram_tensor("wpT", [HD, EMB], BF16, kind="ExternalInput").ap()
    outp = nc.dram_tensor("outp", [N, EMB], F32, kind="ExternalOutput").ap()

    xTr = xT.rearrange("(c p) s -> p c s", p=128)
    wTr = wT.rearrange("(c p) s -> p c s", p=128)
    wpTr = wpT.rearrange("(m p) e -> p m e", p=128)
    outr = outp.rearrange("(s p) e -> p s e", p=128)

    with ExitStack() as persist:
        ppool = persist.enter_context(tc.tile_pool(name="persist", bufs=1))
        wp_sb = ppool.tile([128, MC, EMB], BF16)
        nc.sync.dma_start(wp_sb[:], wpTr)
        qT_sb = ppool.tile([128, MC, N], BF16)
        kT_sb = ppool.tile([128, HL, N], BF16)
        nc.vector.memset(kT_sb[:], 0.0)
        v_sb = ppool.tile([128, NK, HL * (D + 1) + D], BF16)
        nc.vector.memset(v_sb[:], 1.0)
        attT_sb = ppool.tile([128, MC, N], BF16)

        psum_pool = persist.enter_context(
            tc.tile_pool(name="psum", bufs=1, space="PSUM"))

        # ---- phase 1: qkv projection ----
        with ExitStack() as ph1:
            p1 = ph1.enter_context(tc.tile_pool(name="ph1", bufs=1))
            x_sb = p1.tile([128, EC, N], BF16)
            w_sb = p1.tile([128, EC, 3 * HD], BF16)
            for c in range(EC):
                nc.sync.dma_start(w_sb[:, c, :], wTr[:, c, :])
                nc.sync.dma_start(x_sb[:, c, :], xTr[:, c, :])

            for which in (0, 1):
                for m in range(MC):
                    lo = which * HD + m * 128
                    for n in range(NQ):
                        mm = psum_pool.tile([128, 512], F32, tag="av", bufs=4, name=f"mm_{which}_{m}_{n}")
                        for c in range(EC):
                            nc.tensor.matmul(
                                mm[:],
                                (w_sb[:, c, lo:lo + 128]),
                                (x_sb[:, c, n * 512:(n + 1) * 512]),
                                start=(c == 0), stop=(c == EC - 1))
                        ns = slice(n * 512, (n + 1) * 512)
                        if which == 0:
                            nc.vector.tensor_copy(qT_sb[:, m, ns], mm[:])
                        else:
                            nc.vector.tensor_copy(kT_sb[0:64, 2 * m, ns], mm[0:64, :])
                            nc.vector.tensor_copy(kT_sb[64:128, 2 * m + 1, ns], mm[64:128, :])

            for s in range(NK):
                vv = psum_pool.tile([128, 2, 512], F32, tag="eps", bufs=2, name=f"vv_{s}")[:, 0, 0:HD]
                for c in range(EC):
                    nc.tensor.matmul(
                        vv[:],
                        (x_sb[:, c, s * 128:(s + 1) * 128]),
                        (w_sb[:, c, 2 * HD:3 * HD]),
                        start=(c == 0), stop=(c == EC - 1))
                nc.vector.tensor_copy(
                    v_sb[:, s, 0:HL * (D + 1)].rearrange(
                        "p (h c) -> p h c", c=D + 1)[:, :, 0:D],
                    vv[:].rearrange("p (h d) -> p h d", h=HL))

        # ---- phase 2: attention ----
        with ExitStack() as ph2:
            esb_pool = ph2.enter_context(tc.tile_pool(name="esb", bufs=4))
            sm_pool = ph2.enter_context(tc.tile_pool(name="sm", bufs=4))

            for h in range(HL):
                m, p0 = h // 2, (h % 2) * 64
                avs = [psum_pool.tile([128, 512], F32, tag="av", bufs=4, name=f"av_{h}_{n}")
                       for n in range(NQ)]
                for kk in range(NK):
                    # two 2-bank energy tiles per kk so the next group's QK
                    # overlaps this group's exp (keeps the PE array gap-free;
                    # periodic array idles re-throttle the HAM clock gate)
                    e_sbs = []
                    for half in range(2):
                        e_ps = psum_pool.tile([128, 2, 512], F32, tag="eps", bufs=2,
                                             name=f"eps_{h}_{kk}_{half}")
                        for j in range(2):
                            n = half * 2 + j
                            nc.tensor.matmul(
                                e_ps[:, j, :],
                                (kT_sb[:, h, kk * 128:(kk + 1) * 128]),
                                (qT_sb[0:128, m, n * 512:(n + 1) * 512]),
                                start=True, stop=True)
                        e_sb = esb_pool.tile([128, 2, 512], BF16, tag="esb",
                                             name=f"esb_{h}_{kk}_{half}")
                        nc.scalar.activation(e_sb[:], e_ps[:], EXP, scale=SCALE)
                        e_sbs.append(e_sb)
                    for n in range(NQ):
                        nc.tensor.matmul(
                            avs[n][:],
                            (v_sb[:, kk, h * (D + 1): h * (D + 1) + 128]),
                            (e_sbs[n // 2][:, n % 2, :]),
                            start=(kk == 0), stop=(kk == NK - 1))
                # drain all four av banks first (the slow reciprocals would
                # otherwise sit ahead of the copies in the DVE queue and stall
                # the next head's AV accumulation on bank reuse)
                avsts = []
                for n in range(NQ):
                    avst = sm_pool.tile([D + 1, 512], F32, tag="avst", bufs=8,
                                        name=f"avst_{h}_{n}")
                    nc.vector.tensor_copy(avst[:], avs[n][0:D + 1, :])
                    avsts.append(avst)
                for n in range(NQ):
                    rec = sm_pool.tile([1, 512], F32, tag="rec", name=f"rec_{h}_{n}")
                    nc.vector.reciprocal(rec[:], avsts[n][D:D + 1, :])
                    rb = sm_pool.tile([D, 512], F32, tag="rb", name=f"rb_{h}_{n}")
                    nc.gpsimd.partition_broadcast(rb[:], rec[:])
                    nc.vector.tensor_mul(
                        attT_sb[p0:p0 + 64, m, n * 512:(n + 1) * 512],
                        avsts[n][0:D, :], rb[:])

        # ---- phase 3: output projection (natural layout) ----
        with ExitStack() as ph3:
            osb_pool = ph3.enter_context(tc.tile_pool(name="osb", bufs=3))
            for s in range(NK):
                o_sb = osb_pool.tile([128, EMB], F32, tag="osb", name=f"osb_{s}")
                for half in range(2):
                    pr = psum_pool.tile([128, 512], F32, tag="av", bufs=4, name=f"pr_{s}_{half}")[:, 0:HD]
                    for m in range(MC):
                        nc.tensor.matmul(
                            pr[:],
                            (attT_sb[:, m, s * 128:(s + 1) * 128]),
                            (wp_sb[:, m, half * HD:(half + 1) * HD]),
                            start=(m == 0), stop=(m == MC - 1))
                    nc.vector.tensor_copy(o_sb[:, half * HD:(half + 1) * HD], pr[:])
                nc.sync.dma_start(outr[:, s, :], o_sb[:])


_CACHE = {}


def _build():
    if "nc" not in _CACHE:
        nc = bacc.Bacc("TRN2", target_bir_lowering=False, debug=False,
                       num_devices=NCORES)
        with tile.TileContext(nc) as tc:
            _emit(tc)
        nc.compile()
        _CACHE["nc"] = nc
    return _CACHE["nc"]


def _in_maps(x, W_qkv, W_proj):
    in_maps = []
    for c in range(NCORES):
        b, g = divmod(c, 2)
        r0 = g * HD
        w_rows = np.concatenate([
            W_qkv[0 * EMB + r0: 0 * EMB + r0 + HD],
            W_qkv[1 * EMB + r0: 1 * EMB + r0 + HD],
            W_qkv[2 * EMB + r0: 2 * EMB + r0 + HD],
        ], axis=0)                                   # [1152, 768]
        bf = ml_dtypes.bfloat16
        in_maps.append({
            "xT": np.ascontiguousarray(x[b].T.astype(bf)),
            "wT": np.ascontiguousarray(w_rows.T.astype(bf)),
            "wpT": np.ascontiguousarray(W_proj[:, r0:r0 + HD].T.astype(bf)),
        })
    return in_maps


LAST_RESULTS = None


def kernel(x, W_qkv, W_proj, b_proj):
    global LAST_RESULTS
    x = np.ascontiguousarray(np.asarray(x, dtype=np.float32))
    W_qkv = np.asarray(W_qkv, dtype=np.float32)
    W_proj = np.asarray(W_proj, dtype=np.float32)
    b_proj = np.asarray(b_proj, dtype=np.float32)

    nc = _build()
    in_maps = _in_maps(x, W_qkv, W_proj)
    res = run_bass_kernel_spmd(nc, in_maps, core_ids=list(range(NCORES)))
    LAST_RESULTS = res

    out = np.empty((B, N, EMB), dtype=np.float32)
    for b in range(B):
        out[b] = res.results[2 * b]["outp"] + res.results[2 * b + 1]["outp"]
    out += b_proj
    return out
